# revision 19
# baseline (speedup 1.0000x reference)
"""Trainium2 Bass kernel for nn_AttentionBlock (S=2048, DM=1024, H=16, HD=64).

Strategy (8 NeuronCores, tensor-parallel over heads):
  - Each core owns 2 heads (a 128-wide slice of the hidden dim).
  - Host pre-transposes x and the weight shards so every matmul contracts
    over the partition dim with no on-device transposes of activations:
      Q^T/K^T/V^T [hd2=128, S] = W_shard @ x^T   (accumulate 8 dm-chunks)
      logits^T [k, q] = (K^T slice)  ^T-contract  (Q^T)        per head
      P^T = exp(logits/8)  (softmax denominator comes free from a ones
            column appended to V in the P@V matmul)
      attn^T [hd2, S] = V_aug^T-contract P^T, normalized by the denom row
      partial [S, DM] = attn^T ^T-contract Wo^T-shard
  - ReduceScatter(partial) over the 8 cores in 4 chunks of 512 rows
    (overlaps comm with compute), then residual + layernorm on each
    core's 4x64 rows, host reassembles.
All matmuls run in bf16 with f32 PSUM accumulation; the residual path
(x + attn_out) stays f32, which keeps the final error tiny because the
residual dominates the layernorm input.
"""

import numpy as np
import ml_dtypes

import concourse.bass as bass
import concourse.bacc as bacc
import concourse.mybir as mybir
import concourse.tile as tile
from concourse import bass_utils

dt = mybir.dt
AF = mybir.ActivationFunctionType
ALU = mybir.AluOpType

S, DM, H, HD = 2048, 1024, 16, 64
NCORES = 8
HPC = H // NCORES            # heads per core = 2
HD2 = HPC * HD               # 128, hidden slice per core
EPS = 1e-5
NQ = 4                       # q tiles of 512
QT = S // NQ                 # 512
NK = S // 128                # 16 k-chunks of 128
NDM = DM // 128              # 8 dm chunks
ROWS = S // NCORES // NQ     # 64 rows per (core, chunk) after reduce-scatter

BF = dt.bfloat16
F32 = dt.float32


DEBUG_TAPS = False


def _build_program():
    nc = bacc.Bacc("TRN2", target_bir_lowering=False, debug=False,
                   num_devices=NCORES)

    xT_d = nc.dram_tensor("xT", [DM, S], BF, kind="ExternalInput").ap()
    wqT_d = nc.dram_tensor("wqT", [DM, HD2], BF, kind="ExternalInput").ap()
    wkT_d = nc.dram_tensor("wkT", [DM, HD2], BF, kind="ExternalInput").ap()
    wvT_d = nc.dram_tensor("wvT", [DM, HD2], BF, kind="ExternalInput").ap()
    woT_d = nc.dram_tensor("woT", [HD2, DM], BF, kind="ExternalInput").ap()
    biasT_d = nc.dram_tensor("biasT", [HD2, S], F32, kind="ExternalInput").ap()
    xres_d = nc.dram_tensor("xres", [NQ * ROWS, DM], F32, kind="ExternalInput").ap()
    gamma_d = nc.dram_tensor("gamma", [1, DM], F32, kind="ExternalInput").ap()
    beta_d = nc.dram_tensor("beta", [1, DM], F32, kind="ExternalInput").ap()
    out_d = nc.dram_tensor("out", [NQ * ROWS, DM], F32, kind="ExternalOutput").ap()

    with tile.TileContext(nc) as tc:
        _build(tc, xT_d, wqT_d, wkT_d, wvT_d, woT_d, biasT_d, xres_d,
               gamma_d, beta_d, out_d)
    nc.compile()
    return nc


def _build(tc, xT_d, wqT_d, wkT_d, wvT_d, woT_d, biasT_d, xres_d,
           gamma_d, beta_d, out_d):
    nc = tc.nc
    P = 128

    const = tc.alloc_tile_pool(name="const", bufs=1)
    persist = tc.alloc_tile_pool(name="persist", bufs=1)
    ptp = tc.alloc_tile_pool(name="ptp", bufs=3)
    small = tc.alloc_tile_pool(name="small", bufs=2)
    psA = tc.alloc_tile_pool(name="psA", bufs=4, space="PSUM")
    psPV = tc.alloc_tile_pool(name="psPV", bufs=2, space="PSUM")
    dram = tc.alloc_tile_pool(name="dram", bufs=1, space="DRAM")

    # ---- constants / inputs to SBUF ----
    xT_sb = const.tile([P, NDM, S], BF, tag="xT_sb")
    nc.sync.dma_start(xT_sb[:], xT_d.rearrange("(c p) s -> p c s", p=P))
    wq_sb = const.tile([P, NDM, HD2], BF, tag="wq_sb")
    nc.sync.dma_start(wq_sb[:], wqT_d.rearrange("(c p) m -> p c m", p=P))
    wk_sb = const.tile([P, NDM, HD2], BF, tag="wk_sb")
    nc.sync.dma_start(wk_sb[:], wkT_d.rearrange("(c p) m -> p c m", p=P))
    wv_sb = const.tile([P, NDM, HD2], BF, tag="wv_sb")
    nc.sync.dma_start(wv_sb[:], wvT_d.rearrange("(c p) m -> p c m", p=P))
    wo_sb = const.tile([P, DM], BF, tag="wo_sb")
    nc.sync.dma_start(wo_sb[:], woT_d)
    biasT_sb = const.tile([P, S], F32, tag="biasT_sb")
    nc.sync.dma_start(biasT_sb[:], biasT_d)
    gammab = const.tile([P, DM], F32, tag="gammab")
    nc.sync.dma_start(gammab[:], gamma_d.to_broadcast((P, DM)))
    betab = const.tile([P, DM], F32, tag="betab")
    nc.sync.dma_start(betab[:], beta_d.to_broadcast((P, DM)))
    xres_sb = const.tile([ROWS, NQ, DM], F32, tag="xres_sb")
    nc.sync.dma_start(xres_sb[:], xres_d.rearrange("(j r) d -> r j d", r=ROWS))
    eps_sb = const.tile([ROWS, 1], F32, tag="eps_sb")
    nc.vector.memset(eps_sb[:], EPS)

    # ---- persistent activations ----
    qT_sb = persist.tile([P, S], BF, tag="qT_sb")      # Q^T (+bias), 2 heads
    kT_sb = persist.tile([P, S], BF, tag="kT_sb")      # K^T (+bias)
    v_sb = persist.tile([P, NK, 2 * (HD + 1)], BF, tag="v_sb")  # [V0|1|V1|1]
    attnT_sb = persist.tile([P, S], BF, tag="attnT_sb")

    # ---- projections: Q^T/K^T [hd2, S] = W_shard @ x^T ----
    for w, dst in ((wq_sb, qT_sb), (wk_sb, kT_sb)):
        for j in range(NQ):
            js = slice(j * QT, (j + 1) * QT)
            ps = psA.tile([P, QT], F32, tag="mm", name="ps")
            for c in range(NDM):
                nc.tensor.matmul(ps[:], lhsT=w[:, c, :],
                                 rhs=xT_sb[:, c, js],
                                 start=(c == 0), stop=(c == NDM - 1))
            nc.vector.tensor_add(dst[:, js], ps[:], biasT_sb[:, js])

    # ---- V directly in [s, hd] layout: V = x @ Wv_shard^T ----
    # v_sb free layout per k-chunk: [V0 (64) | 1 | V1 (64) | 1]
    for t in range(NK):
        ts = slice(t * P, (t + 1) * P)
        psv = psA.tile([P, P], F32, tag="mmv", name="psv", bufs=2)
        for c in range(NDM):
            nc.tensor.matmul(psv[:], lhsT=xT_sb[:, c, ts], rhs=wv_sb[:, c, :],
                             start=(c == 0), stop=(c == NDM - 1))
        nc.vector.tensor_copy(v_sb[:, t, 0:HD], psv[:, 0:HD])
        nc.vector.tensor_copy(v_sb[:, t, HD + 1:2 * HD + 1], psv[:, HD:2 * HD])
    nc.vector.memset(v_sb[:, :, HD:HD + 1], 1.0)
    nc.vector.memset(v_sb[:, :, 2 * HD + 1:2 * HD + 2], 1.0)

    partial_d = dram.tile([S, DM], F32, tag="partial_d")
    rs_tiles = []
    for j in range(NQ):
        rs_j = dram.tile([ROWS, DM], F32, tag=f"rs_{j}", name=f"rs_{j}")
        rs_tiles.append(rs_j)

    # ---- attention per q-block j, heads inner ----
    inv_sqrt_hd = 1.0 / np.sqrt(HD)
    for j in range(NQ):
        js = slice(j * QT, (j + 1) * QT)
        for h in range(HPC):
            hs = slice(h * HD, (h + 1) * HD)
            pv = psPV.tile([HD + 1, QT], F32, tag="pv", name="pv")
            for ki in range(NK):
                ks = slice(ki * P, (ki + 1) * P)
                lg = psA.tile([P, QT], F32, tag="mm", name="lg")
                nc.tensor.matmul(lg[:], lhsT=kT_sb[hs, ks], rhs=qT_sb[hs, js],
                                 start=True, stop=True)
                pt = ptp.tile([P, QT], BF, tag="pt", name="pt")
                nc.scalar.activation(pt[:], lg[:], AF.Exp, scale=inv_sqrt_hd)
                vcol = slice(h * (HD + 1), (h + 1) * (HD + 1))
                nc.tensor.matmul(pv[:], lhsT=v_sb[:, ki, vcol], rhs=pt[:],
                                 start=(ki == 0), stop=(ki == NK - 1))
            # normalize by the denominator row (row HD of pv)
            sbd = small.tile([HD + 1, QT], F32, tag="sbd", name="sbd")
            nc.vector.reciprocal(sbd[HD:HD + 1, :], pv[HD:HD + 1, :])
            drec = dram.tile([1, QT], F32, tag="drec", name="drec", bufs=2)
            nc.sync.dma_start(drec[:], sbd[HD:HD + 1, :])
            rb = small.tile([HD, QT], F32, tag="rb", name="rb")
            nc.sync.dma_start(rb[:], drec.to_broadcast((HD, QT)))
            if h == 0:
                nc.vector.tensor_tensor(out=attnT_sb[0:HD, js],
                                        in0=pv[0:HD, :], in1=rb[:],
                                        op=ALU.mult)
            else:
                h1t = small.tile([HD, QT], BF, tag="h1t", name="h1t")
                nc.vector.tensor_tensor(out=h1t[:], in0=pv[0:HD, :],
                                        in1=rb[:], op=ALU.mult)
                nc.sync.dma_start(attnT_sb[HD:2 * HD, js], h1t[:])

        # ---- output projection for this q-block ----
        for t in range(j * NQ, (j + 1) * NQ):
            ts = slice(t * P, (t + 1) * P)
            for n in range(DM // QT):
                ns = slice(n * QT, (n + 1) * QT)
                po = psA.tile([P, QT], F32, tag="mm", name="po")
                nc.tensor.matmul(po[:], lhsT=attnT_sb[:, ts], rhs=wo_sb[:, ns],
                                 start=True, stop=True)
                posb = small.tile([P, QT], F32, tag="posb", name="posb",
                                  bufs=3)
                nc.vector.tensor_copy(posb[:], po[:])
                nc.sync.dma_start(partial_d[ts, ns], posb[:])

        # ---- reduce-scatter this 512-row block across the 8 cores ----
        nc.gpsimd.collective_compute(
            "ReduceScatter", ALU.add,
            replica_groups=[list(range(NCORES))],
            ins=[partial_d[js, :].opt()],
            outs=[rs_tiles[j][:].opt()],
        )

        # ---- residual + layernorm on our 64 rows of this block ----
        rs_sb = small.tile([ROWS, DM], F32, tag="rs_sb", name="rs_sb")
        nc.sync.dma_start(rs_sb[:], rs_tiles[j][:])
        res = small.tile([ROWS, DM], F32, tag="res", name="res")
        nc.vector.tensor_add(res[:], rs_sb[:], xres_sb[:, j, :])
        sums = small.tile([ROWS, 1], F32, tag="sums", name="sums")
        nc.vector.reduce_sum(sums[:], res[:], axis=mybir.AxisListType.X)
        neg_mu = small.tile([ROWS, 1], F32, tag="neg_mu", name="neg_mu")
        nc.vector.tensor_scalar_mul(neg_mu[:], sums[:], -1.0 / DM)
        sq = small.tile([ROWS, DM], F32, tag="sq", name="sq")
        sumsq = small.tile([ROWS, 1], F32, tag="sumsq", name="sumsq")
        nc.scalar.activation(sq[:], res[:], AF.Square, accum_out=sumsq[:])
        # var = sumsq/DM - mu^2
        mu2 = small.tile([ROWS, 1], F32, tag="mu2", name="mu2")
        nc.vector.tensor_tensor(out=mu2[:], in0=neg_mu[:], in1=neg_mu[:],
                                op=ALU.mult)
        var = small.tile([ROWS, 1], F32, tag="var", name="var")
        nc.vector.tensor_scalar_mul(var[:], sumsq[:], 1.0 / DM)
        nc.vector.tensor_tensor(out=var[:], in0=var[:], in1=mu2[:],
                                op=ALU.subtract)
        std = small.tile([ROWS, 1], F32, tag="std", name="std")
        nc.scalar.activation(std[:], var[:], AF.Sqrt, bias=eps_sb[:])
        rstd = small.tile([ROWS, 1], F32, tag="rstd", name="rstd")
        nc.vector.reciprocal(rstd[:], std[:])
        lnb = small.tile([ROWS, 1], F32, tag="lnb", name="lnb")
        nc.vector.tensor_tensor(out=lnb[:], in0=neg_mu[:], in1=rstd[:],
                                op=ALU.mult)
        t1 = small.tile([ROWS, DM], F32, tag="t1", name="t1")
        nc.scalar.activation(t1[:], res[:], AF.Identity, scale=rstd[:],
                             bias=lnb[:])
        t2 = small.tile([ROWS, DM], F32, tag="t2", name="t2")
        nc.vector.tensor_tensor(out=t2[:], in0=t1[:], in1=gammab[0:ROWS, :],
                                op=ALU.mult)
        outt = small.tile([ROWS, DM], F32, tag="outt", name="outt")
        nc.vector.tensor_tensor(out=outt[:], in0=t2[:], in1=betab[0:ROWS, :],
                                op=ALU.add)
        nc.sync.dma_start(out_d[j * ROWS:(j + 1) * ROWS, :], outt[:])

    if DEBUG_TAPS:
        taps = {
            "dbg_qT": qT_sb, "dbg_kT": kT_sb,
            "dbg_vaug": v_sb, "dbg_attnT": attnT_sb,
        }
        for nm, t in taps.items():
            d = nc.dram_tensor(nm, list(t.shape), t.dtype,
                               kind="ExternalOutput").ap()
            nc.sync.dma_start(d[:], t[:])
        for nm, t in (("dbg_partial", partial_d), ("dbg_rs0", rs_tiles[0])):
            d = nc.dram_tensor(nm, list(t.shape), t.dtype,
                               kind="ExternalOutput").ap()
            nc.sync.dma_start(d[:], t[:])

    for pool in (dram, psPV, psA, small, ptp, persist, const):
        pool.release()


_NC_CACHE = None


def _get_program():
    global _NC_CACHE
    if _NC_CACHE is None:
        _NC_CACHE = _build_program()
    return _NC_CACHE


def _prep_inputs(x, static_bias, Wq, Wk, Wv, Wo, ln_gamma, ln_beta):
    bf = ml_dtypes.bfloat16
    x = np.asarray(x, np.float32)
    static_bias = np.asarray(static_bias, np.float32)
    Wq, Wk, Wv, Wo = (np.asarray(w, np.float32) for w in (Wq, Wk, Wv, Wo))
    gamma = np.ascontiguousarray(np.asarray(ln_gamma, np.float32).reshape(1, DM))
    beta = np.ascontiguousarray(np.asarray(ln_beta, np.float32).reshape(1, DM))
    xT = np.ascontiguousarray(x.T).astype(bf)
    in_maps = []
    for c in range(NCORES):
        hs = slice(c * HD2, (c + 1) * HD2)
        wqT = np.ascontiguousarray(Wq[hs, :].T).astype(bf)
        wkT = np.ascontiguousarray(Wk[hs, :].T).astype(bf)
        wvT = np.ascontiguousarray(Wv[hs, :].T).astype(bf)
        woT = np.ascontiguousarray(Wo[:, hs].T).astype(bf)
        biasT = np.ascontiguousarray(
            static_bias[:, c * HPC:(c + 1) * HPC, :].reshape(S, HD2).T)
        xres = np.concatenate(
            [x[j * QT + c * ROWS: j * QT + (c + 1) * ROWS, :]
             for j in range(NQ)], axis=0)
        in_maps.append({
            "xT": xT, "wqT": wqT, "wkT": wkT, "wvT": wvT, "woT": woT,
            "biasT": biasT, "xres": np.ascontiguousarray(xres),
            "gamma": gamma, "beta": beta,
        })
    return in_maps


def _assemble(results):
    out = np.empty((S, DM), np.float32)
    for c in range(NCORES):
        oc = results[c]["out"]
        for j in range(NQ):
            out[j * QT + c * ROWS: j * QT + (c + 1) * ROWS, :] = \
                oc[j * ROWS:(j + 1) * ROWS, :]
    return out


def kernel(x, static_bias, Wq, Wk, Wv, Wo, ln_gamma, ln_beta, mask=None,
           **_ignored):
    nc = _get_program()
    in_maps = _prep_inputs(x, static_bias, Wq, Wk, Wv, Wo, ln_gamma, ln_beta)
    res = bass_utils.run_bass_kernel_spmd(nc, in_maps,
                                          core_ids=list(range(NCORES)))
    return _assemble(res.results)


if __name__ == "__main__":
    import reference
    inputs = {k: np.asarray(v) for k, v in reference.setup_inputs().items()}
    expected = np.asarray(reference.reference(**inputs))
    actual = kernel(**inputs)
    err = np.abs(actual - expected)
    denom = np.abs(expected).max()
    print("absmax err:", err.max(), "rel:", err.max() / denom)


# revision 26
# speedup vs baseline: 1.0458x; 1.0458x over previous
"""Trainium2 Bass kernel for nn_AttentionBlock (S=2048, DM=1024, H=16, HD=64).

Strategy (8 NeuronCores, tensor-parallel over heads):
  - Each core owns 2 heads (a 128-wide slice of the hidden dim).
  - Host pre-transposes x and the weight shards so every matmul contracts
    over the partition dim with no on-device transposes of activations:
      Q^T/K^T [hd2=128, S] = W_shard @ x^T   (accumulate 8 dm-chunks)
      V       [S, hd2]     = x @ Wv_shard^T  (ones columns appended)
      logits^T [k, q] = (K^T slice) x (Q^T)  per head
      P^T = exp(logits/8)  (softmax denominator comes free from a ones
            column appended to V in the P@V matmul)
      attn^T [hd2, S] = V_aug x P^T, normalized by the denominator row
  - AllToAll redistributes attn^T (bf16, 256KB/core per q-superblock)
    so each core holds all 16 heads for its own token slice, then each
    core does the full output projection + residual + layernorm for its
    tokens; host reassembles. Comm is 16x smaller than reduce-scattering
    the f32 partials and the projection needs no collective afterwards.
  - Attention runs on 2 q-superblocks of 1024 so exp() batches into
    N=1024 ACT ops (amortizing the 352-elem fixed cost) while the first
    superblock's AllToAll/projection still overlaps the second's compute.
All matmuls run in bf16 with f32 PSUM accumulation; the residual path
(x + attn_out) stays f32, which keeps the final error tiny because the
residual dominates the layernorm input.
"""

import numpy as np
import ml_dtypes

import concourse.bass as bass
import concourse.bacc as bacc
import concourse.mybir as mybir
import concourse.tile as tile
from concourse import bass_utils

dt = mybir.dt
AF = mybir.ActivationFunctionType
ALU = mybir.AluOpType

S, DM, H, HD = 2048, 1024, 16, 64
NCORES = 8
HPC = H // NCORES            # heads per core = 2
HD2 = HPC * HD               # 128, hidden slice per core
EPS = 1e-5
NJ = 2                       # q superblocks
JW = S // NJ                 # 1024 q per superblock
NK = S // 128                # 16 k-chunks of 128
NDM = DM // 128              # 8 dm chunks
TOK = S // NCORES // NJ      # 128 tokens per (core, superblock)

BF = dt.bfloat16
F32 = dt.float32

DEBUG_TAPS = False
FAKE_A2A = False


def _build_program():
    nc = bacc.Bacc("TRN2", target_bir_lowering=False, debug=False,
                   num_devices=NCORES)

    xT_d = nc.dram_tensor("xT", [DM, S], BF, kind="ExternalInput").ap()
    wqT_d = nc.dram_tensor("wqT", [DM, HD2], BF, kind="ExternalInput").ap()
    wkT_d = nc.dram_tensor("wkT", [DM, HD2], BF, kind="ExternalInput").ap()
    wvT_d = nc.dram_tensor("wvT", [DM, HD2], BF, kind="ExternalInput").ap()
    woF_d = nc.dram_tensor("woF", [NDM, 128, DM], BF, kind="ExternalInput").ap()
    biasT_d = nc.dram_tensor("biasT", [HD2, S], F32, kind="ExternalInput").ap()
    xres_d = nc.dram_tensor("xres", [NJ * TOK, DM], F32, kind="ExternalInput").ap()
    gamma_d = nc.dram_tensor("gamma", [1, DM], F32, kind="ExternalInput").ap()
    beta_d = nc.dram_tensor("beta", [1, DM], F32, kind="ExternalInput").ap()
    out_d = nc.dram_tensor("out", [NJ * TOK, DM], F32, kind="ExternalOutput").ap()

    with tile.TileContext(nc) as tc:
        _build(tc, xT_d, wqT_d, wkT_d, wvT_d, woF_d, biasT_d, xres_d,
               gamma_d, beta_d, out_d)
    nc.compile()
    return nc


def _build(tc, xT_d, wqT_d, wkT_d, wvT_d, woF_d, biasT_d, xres_d,
           gamma_d, beta_d, out_d):
    nc = tc.nc
    P = 128

    const = tc.alloc_tile_pool(name="const", bufs=1)
    persist = tc.alloc_tile_pool(name="persist", bufs=1)
    ptp = tc.alloc_tile_pool(name="ptp", bufs=3)
    small = tc.alloc_tile_pool(name="small", bufs=2)
    psA = tc.alloc_tile_pool(name="psA", bufs=2, space="PSUM")
    psPV = tc.alloc_tile_pool(name="psPV", bufs=2, space="PSUM")
    dram = tc.alloc_tile_pool(name="dram", bufs=1, space="DRAM")

    # ---- constants / inputs to SBUF ----
    xT_sb = const.tile([P, NDM, S], BF, tag="xT_sb")
    nc.sync.dma_start(xT_sb[:], xT_d.rearrange("(c p) s -> p c s", p=P))
    wq_sb = const.tile([P, NDM, HD2], BF, tag="wq_sb")
    nc.sync.dma_start(wq_sb[:], wqT_d.rearrange("(c p) m -> p c m", p=P))
    wk_sb = const.tile([P, NDM, HD2], BF, tag="wk_sb")
    nc.sync.dma_start(wk_sb[:], wkT_d.rearrange("(c p) m -> p c m", p=P))
    wv_sb = const.tile([P, NDM, HD2], BF, tag="wv_sb")
    nc.sync.dma_start(wv_sb[:], wvT_d.rearrange("(c p) m -> p c m", p=P))
    woF_sb = const.tile([P, NDM, DM], BF, tag="woF_sb")
    nc.sync.dma_start(woF_sb[:], woF_d.rearrange("c p d -> p c d"))
    biasT_sb = const.tile([P, S], F32, tag="biasT_sb")
    nc.sync.dma_start(biasT_sb[:], biasT_d)
    gammab = const.tile([P, DM], F32, tag="gammab")
    nc.sync.dma_start(gammab[:], gamma_d.to_broadcast((P, DM)))
    betab = const.tile([P, DM], F32, tag="betab")
    nc.sync.dma_start(betab[:], beta_d.to_broadcast((P, DM)))
    xres_sb = const.tile([TOK, NJ, DM], F32, tag="xres_sb")
    nc.sync.dma_start(xres_sb[:], xres_d.rearrange("(j r) d -> r j d", r=TOK))
    eps_sb = const.tile([P, 1], F32, tag="eps_sb")
    nc.vector.memset(eps_sb[:], EPS)

    # ---- persistent activations ----
    qT_sb = persist.tile([P, S], BF, tag="qT_sb")      # Q^T (+bias), 2 heads
    kT_sb = persist.tile([P, S], BF, tag="kT_sb")      # K^T (+bias)
    v_sb = persist.tile([P, NK, 2 * (HD + 1)], BF, tag="v_sb")  # [V0|1|V1|1]
    attnT_sb = persist.tile([P, S], BF, tag="attnT_sb")

    # ---- projections: Q^T/K^T [hd2, S] = W_shard @ x^T ----
    for w, dst in ((wk_sb, kT_sb), (wq_sb, qT_sb)):
        for j in range(NJ):
            ps = psA.tile([P, JW], F32, tag="mm", name="ps")
            for half in range(JW // 512):
                q0 = j * JW + half * 512
                for c in range(NDM):
                    nc.tensor.matmul(ps[:, half * 512:(half + 1) * 512],
                                     lhsT=w[:, c, :],
                                     rhs=xT_sb[:, c, q0:q0 + 512],
                                     start=(c == 0), stop=(c == NDM - 1))
            nc.vector.tensor_add(dst[:, j * JW:(j + 1) * JW], ps[:],
                                 biasT_sb[:, j * JW:(j + 1) * JW])

    # ---- V in [s, hd] layout: V = x @ Wv_shard^T; ones cols appended ----
    for t in range(NK):
        ts = slice(t * P, (t + 1) * P)
        psv = psA.tile([P, JW], F32, tag="mm", name="psv")
        for c in range(NDM):
            nc.tensor.matmul(psv[:, 0:P], lhsT=xT_sb[:, c, ts],
                             rhs=wv_sb[:, c, :],
                             start=(c == 0), stop=(c == NDM - 1))
        nc.vector.tensor_copy(v_sb[:, t, 0:HD], psv[:, 0:HD])
        nc.vector.tensor_copy(v_sb[:, t, HD + 1:2 * HD + 1], psv[:, HD:2 * HD])
    nc.vector.memset(v_sb[:, :, HD:HD + 1], 1.0)
    nc.vector.memset(v_sb[:, :, 2 * HD + 1:2 * HD + 2], 1.0)

    # AllGather bounce buffers (bf16): in = my heads' attn block,
    # out = [src core, hd-slice, q of block]
    ag_in, ag_out = [], []
    for j in range(NJ):
        ag_in_j = dram.tile([P, JW], BF, tag=f"ag_in_{j}", name=f"ag_in_{j}")
        ag_out_j = dram.tile([NCORES, P, JW], BF, tag=f"ag_out_{j}",
                             name=f"ag_out_{j}", addr_space="Shared")
        ag_in.append(ag_in_j)
        ag_out.append(ag_out_j)

    inv_sqrt_hd = float(1.0 / np.sqrt(HD))
    for j in range(NJ):
        js = slice(j * JW, (j + 1) * JW)
        # ---- attention for this q-superblock, per head ----
        for h in range(HPC):
            hs = slice(h * HD, (h + 1) * HD)
            pv = psPV.tile([HD + 1, JW], F32, tag="pv", name="pv")
            for ki in range(NK):
                ks = slice(ki * P, (ki + 1) * P)
                lg = psA.tile([P, JW], F32, tag="mm", name="lg")
                for half in range(JW // 512):
                    q0 = j * JW + half * 512
                    nc.tensor.matmul(lg[:, half * 512:(half + 1) * 512],
                                     lhsT=kT_sb[hs, ks],
                                     rhs=qT_sb[hs, q0:q0 + 512],
                                     start=True, stop=True)
                pt = ptp.tile([P, JW], BF, tag="pt", name="pt")
                nc.scalar.activation(pt[:], lg[:], AF.Exp, scale=inv_sqrt_hd)
                vcol = slice(h * (HD + 1), (h + 1) * (HD + 1))
                for half in range(JW // 512):
                    nc.tensor.matmul(pv[:, half * 512:(half + 1) * 512],
                                     lhsT=v_sb[:, ki, vcol],
                                     rhs=pt[:, half * 512:(half + 1) * 512],
                                     start=(ki == 0), stop=(ki == NK - 1))
            # normalize: denom row -> DRAM -> broadcast -> recip -> mult
            sbd = small.tile([HD + 1, JW], F32, tag="sbd", name="sbd")
            nc.vector.tensor_copy(sbd[HD:HD + 1, :], pv[HD:HD + 1, :])
            drec = dram.tile([1, JW], F32, tag="drec", name="drec", bufs=2)
            nc.sync.dma_start(drec[:], sbd[HD:HD + 1, :])
            rb = small.tile([HD, JW], F32, tag="rb", name="rb")
            nc.sync.dma_start(rb[:], drec.to_broadcast((HD, JW)))
            rc = small.tile([HD, JW], F32, tag="rc", name="rc")
            nc.vector.reciprocal(rc[:], rb[:])
            if h == 0:
                nc.vector.tensor_tensor(out=attnT_sb[0:HD, js],
                                        in0=pv[0:HD, :], in1=rc[:],
                                        op=ALU.mult)
            else:
                h1t = small.tile([HD, JW], BF, tag="h1t", name="h1t")
                nc.vector.tensor_tensor(out=h1t[:], in0=pv[0:HD, :],
                                        in1=rc[:], op=ALU.mult)
                nc.sync.dma_start(attnT_sb[HD:2 * HD, js], h1t[:])

        # ---- AllGather heads, then pick our token slice dynamically ----
        nc.sync.dma_start(ag_in[j][:], attnT_sb[:, js])
        if FAKE_A2A:
            nc.sync.dma_start(ag_out[j][0, :, :], ag_in[j][:])
        else:
            nc.gpsimd.collective_compute(
                "AllGather", ALU.bypass,
                replica_groups=[list(range(NCORES))],
                ins=[ag_in[j][:].opt()],
                outs=[ag_out[j][:].opt()],
            )
        afull = small.tile([P, NCORES, TOK], BF, tag="afull", name="afull")
        pid = nc.sync.partition_id()
        ag_v = ag_out[j].rearrange("c p (u t) -> p c u t", u=NCORES)
        nc.sync.dma_start(afull[:], ag_v[:, :, bass.ds(pid, 1), :])

        # ---- full output projection for our TOK tokens of block j ----
        po = psA.tile([P, DM], F32, tag="mm", name="po")
        for n in range(DM // 512):
            ns = slice(n * 512, (n + 1) * 512)
            for c in range(NDM):
                nc.tensor.matmul(po[:, ns], lhsT=afull[:, c, :],
                                 rhs=woF_sb[:, c, ns],
                                 start=(c == 0), stop=(c == NDM - 1))

        # ---- residual + layernorm ----
        res = small.tile([P, DM], F32, tag="res", name="res")
        nc.vector.tensor_add(res[:], po[:], xres_sb[:, j, :])
        sums = small.tile([P, 1], F32, tag="sums", name="sums")
        nc.vector.reduce_sum(sums[:], res[:], axis=mybir.AxisListType.X)
        neg_mu = small.tile([P, 1], F32, tag="neg_mu", name="neg_mu")
        nc.vector.tensor_scalar_mul(neg_mu[:], sums[:], -1.0 / DM)
        sq = small.tile([P, DM], F32, tag="sq", name="sq")
        sumsq = small.tile([P, 1], F32, tag="sumsq", name="sumsq")
        nc.scalar.activation(sq[:], res[:], AF.Square, accum_out=sumsq[:])
        mu2 = small.tile([P, 1], F32, tag="mu2", name="mu2")
        nc.vector.tensor_tensor(out=mu2[:], in0=neg_mu[:], in1=neg_mu[:],
                                op=ALU.mult)
        var = small.tile([P, 1], F32, tag="var", name="var")
        nc.vector.tensor_scalar_mul(var[:], sumsq[:], 1.0 / DM)
        nc.vector.tensor_tensor(out=var[:], in0=var[:], in1=mu2[:],
                                op=ALU.subtract)
        std = small.tile([P, 1], F32, tag="std", name="std")
        nc.scalar.activation(std[:], var[:], AF.Sqrt, bias=eps_sb[:])
        rstd = small.tile([P, 1], F32, tag="rstd", name="rstd")
        nc.vector.reciprocal(rstd[:], std[:])
        lnb = small.tile([P, 1], F32, tag="lnb", name="lnb")
        nc.vector.tensor_tensor(out=lnb[:], in0=neg_mu[:], in1=rstd[:],
                                op=ALU.mult)
        t1 = small.tile([P, DM], F32, tag="t1", name="t1")
        nc.scalar.activation(t1[:], res[:], AF.Identity, scale=rstd[:],
                             bias=lnb[:])
        t2 = small.tile([P, DM], F32, tag="t2", name="t2")
        nc.vector.tensor_tensor(out=t2[:], in0=t1[:], in1=gammab[:],
                                op=ALU.mult)
        outt = small.tile([P, DM], F32, tag="outt", name="outt")
        nc.vector.tensor_tensor(out=outt[:], in0=t2[:], in1=betab[:],
                                op=ALU.add)
        nc.sync.dma_start(out_d[j * TOK:(j + 1) * TOK, :], outt[:])

    if DEBUG_TAPS:
        taps = {
            "dbg_qT": qT_sb, "dbg_kT": kT_sb,
            "dbg_vaug": v_sb, "dbg_attnT": attnT_sb,
        }
        for nm, t in taps.items():
            d = nc.dram_tensor(nm, list(t.shape), t.dtype,
                               kind="ExternalOutput").ap()
            nc.sync.dma_start(d[:], t[:])
        for j in range(NJ):
            d = nc.dram_tensor(f"dbg_a2a_{j}", list(a2a_out[j].shape), BF,
                               kind="ExternalOutput").ap()
            nc.sync.dma_start(d[:], a2a_out[j][:])

    for pool in (dram, psPV, psA, small, ptp, persist, const):
        pool.release()


_NC_CACHE = None


def _get_program():
    global _NC_CACHE
    if _NC_CACHE is None:
        _NC_CACHE = _build_program()
    return _NC_CACHE


def _token_rows(core):
    """Global token indices owned by `core`, in device output order."""
    rows = []
    for j in range(NJ):
        start = j * JW + core * TOK
        rows.extend(range(start, start + TOK))
    return np.array(rows)


def _prep_inputs(x, static_bias, Wq, Wk, Wv, Wo, ln_gamma, ln_beta):
    bf = ml_dtypes.bfloat16
    x = np.asarray(x, np.float32)
    static_bias = np.asarray(static_bias, np.float32)
    Wq, Wk, Wv, Wo = (np.asarray(w, np.float32) for w in (Wq, Wk, Wv, Wo))
    gamma = np.ascontiguousarray(np.asarray(ln_gamma, np.float32).reshape(1, DM))
    beta = np.ascontiguousarray(np.asarray(ln_beta, np.float32).reshape(1, DM))
    xT = np.ascontiguousarray(x.T).astype(bf)
    woF = np.ascontiguousarray(Wo.T.reshape(NDM, 128, DM)).astype(bf)
    in_maps = []
    for c in range(NCORES):
        hs = slice(c * HD2, (c + 1) * HD2)
        wqT = np.ascontiguousarray(Wq[hs, :].T).astype(bf)
        wkT = np.ascontiguousarray(Wk[hs, :].T).astype(bf)
        wvT = np.ascontiguousarray(Wv[hs, :].T).astype(bf)
        biasT = np.ascontiguousarray(
            static_bias[:, c * HPC:(c + 1) * HPC, :].reshape(S, HD2).T)
        xres = np.ascontiguousarray(x[_token_rows(c), :])
        in_maps.append({
            "xT": xT, "wqT": wqT, "wkT": wkT, "wvT": wvT, "woF": woF,
            "biasT": biasT, "xres": xres, "gamma": gamma, "beta": beta,
        })
    return in_maps


def _assemble(results):
    out = np.empty((S, DM), np.float32)
    for c in range(NCORES):
        out[_token_rows(c), :] = results[c]["out"]
    return out


def kernel(x, static_bias, Wq, Wk, Wv, Wo, ln_gamma, ln_beta, mask=None,
           **_ignored):
    nc = _get_program()
    in_maps = _prep_inputs(x, static_bias, Wq, Wk, Wv, Wo, ln_gamma, ln_beta)
    res = bass_utils.run_bass_kernel_spmd(nc, in_maps,
                                          core_ids=list(range(NCORES)))
    return _assemble(res.results)


if __name__ == "__main__":
    import reference
    inputs = {k: np.asarray(v) for k, v in reference.setup_inputs().items()}
    expected = np.asarray(reference.reference(**inputs))
    actual = kernel(**inputs)
    err = np.abs(actual - expected)
    denom = np.abs(expected).max()
    print("absmax err:", err.max(), "rel:", err.max() / denom)


# revision 30
# speedup vs baseline: 1.3734x; 1.3132x over previous
"""Trainium2 Bass kernel for nn_AttentionBlock (S=2048, DM=1024, H=16, HD=64).

Strategy (8 NeuronCores, tensor-parallel over heads):
  - Each core owns 2 heads (a 128-wide slice of the hidden dim).
  - Host pre-transposes x and the weight shards so every matmul contracts
    over the partition dim with no on-device transposes of activations:
      Q^T/K^T [hd2=128, S] = W_shard @ x^T   (accumulate 8 dm-chunks)
      V       [S, hd2]     = x @ Wv_shard^T  (ones columns appended)
      logits^T [k, q] = (K^T slice) x (Q^T)  per head
      P^T = exp(logits/8)  (softmax denominator comes free from a ones
            column appended to V in the P@V matmul)
      attn^T [hd2, S] = V_aug x P^T, normalized by the denominator row
  - AllToAll redistributes attn^T (bf16, 256KB/core per q-superblock)
    so each core holds all 16 heads for its own token slice, then each
    core does the full output projection + residual + layernorm for its
    tokens; host reassembles. Comm is 16x smaller than reduce-scattering
    the f32 partials and the projection needs no collective afterwards.
  - Attention runs on 2 q-superblocks of 1024 so exp() batches into
    N=1024 ACT ops (amortizing the 352-elem fixed cost) while the first
    superblock's AllToAll/projection still overlaps the second's compute.
All matmuls run in bf16 with f32 PSUM accumulation; the residual path
(x + attn_out) stays f32, which keeps the final error tiny because the
residual dominates the layernorm input.
"""

import numpy as np
import ml_dtypes

import concourse.bass as bass
import concourse.bacc as bacc
import concourse.mybir as mybir
import concourse.tile as tile
from concourse import bass_utils

dt = mybir.dt
AF = mybir.ActivationFunctionType
ALU = mybir.AluOpType

S, DM, H, HD = 2048, 1024, 16, 64
NCORES = 8
HPC = H // NCORES            # heads per core = 2
HD2 = HPC * HD               # 128, hidden slice per core
EPS = 1e-5
NJ = 2                       # q superblocks
JW = S // NJ                 # 1024 q per superblock
NK = S // 128                # 16 k-chunks of 128
NDM = DM // 128              # 8 dm chunks
TOK = S // NCORES // NJ      # 128 tokens per (core, superblock)

BF = dt.bfloat16
F32 = dt.float32

DEBUG_TAPS = False
FAKE_A2A = False


def _build_program():
    nc = bacc.Bacc("TRN2", target_bir_lowering=False, debug=False,
                   num_devices=NCORES)

    xT_d = nc.dram_tensor("xT", [DM, S], BF, kind="ExternalInput").ap()
    wqT_d = nc.dram_tensor("wqT", [DM, HD2], BF, kind="ExternalInput").ap()
    wkT_d = nc.dram_tensor("wkT", [DM, HD2], BF, kind="ExternalInput").ap()
    wvT_d = nc.dram_tensor("wvT", [DM, HD2], BF, kind="ExternalInput").ap()
    woF_d = nc.dram_tensor("woF", [NDM, 128, DM], BF, kind="ExternalInput").ap()
    biasT_d = nc.dram_tensor("biasT", [HD2, S], F32, kind="ExternalInput").ap()
    xres_d = nc.dram_tensor("xres", [NJ * TOK, DM], F32, kind="ExternalInput").ap()
    gamma_d = nc.dram_tensor("gamma", [1, DM], F32, kind="ExternalInput").ap()
    beta_d = nc.dram_tensor("beta", [1, DM], F32, kind="ExternalInput").ap()
    out_d = nc.dram_tensor("out", [NJ * TOK, DM], F32, kind="ExternalOutput").ap()

    with tile.TileContext(nc) as tc:
        _build(tc, xT_d, wqT_d, wkT_d, wvT_d, woF_d, biasT_d, xres_d,
               gamma_d, beta_d, out_d)
    nc.compile()
    return nc


def _build(tc, xT_d, wqT_d, wkT_d, wvT_d, woF_d, biasT_d, xres_d,
           gamma_d, beta_d, out_d):
    nc = tc.nc
    P = 128

    const = tc.alloc_tile_pool(name="const", bufs=1)
    persist = tc.alloc_tile_pool(name="persist", bufs=1)
    ptp = tc.alloc_tile_pool(name="ptp", bufs=3)
    small = tc.alloc_tile_pool(name="small", bufs=2)
    psA = tc.alloc_tile_pool(name="psA", bufs=2, space="PSUM")
    psPV = tc.alloc_tile_pool(name="psPV", bufs=1, space="PSUM")
    dram = tc.alloc_tile_pool(name="dram", bufs=1, space="DRAM")

    # ---- constants / inputs to SBUF ----
    xT_sb = const.tile([P, NDM, S], BF, tag="xT_sb")
    xT_v = xT_d.rearrange("(c p) s -> p c s", p=P)
    for c in range(NDM):
        nc.sync.dma_start(xT_sb[:, c, :], xT_v[:, c, :])
    wq_sb = const.tile([P, NDM, HD2], BF, tag="wq_sb")
    nc.sync.dma_start(wq_sb[:], wqT_d.rearrange("(c p) m -> p c m", p=P))
    wk_sb = const.tile([P, NDM, HD2], BF, tag="wk_sb")
    nc.sync.dma_start(wk_sb[:], wkT_d.rearrange("(c p) m -> p c m", p=P))
    wv_sb = const.tile([P, NDM, HD2], BF, tag="wv_sb")
    nc.sync.dma_start(wv_sb[:], wvT_d.rearrange("(c p) m -> p c m", p=P))
    woF_sb = const.tile([P, NDM, DM], BF, tag="woF_sb")
    nc.sync.dma_start(woF_sb[:], woF_d.rearrange("c p d -> p c d"))
    biasT_sb = const.tile([P, S], F32, tag="biasT_sb")
    nc.sync.dma_start(biasT_sb[:], biasT_d)
    gammab = const.tile([P, DM], F32, tag="gammab")
    nc.sync.dma_start(gammab[:], gamma_d.to_broadcast((P, DM)))
    betab = const.tile([P, DM], F32, tag="betab")
    nc.sync.dma_start(betab[:], beta_d.to_broadcast((P, DM)))
    xres_sb = const.tile([TOK, NJ, DM], F32, tag="xres_sb")
    nc.sync.dma_start(xres_sb[:], xres_d.rearrange("(j r) d -> r j d", r=TOK))
    eps_sb = const.tile([P, 1], F32, tag="eps_sb")
    nc.vector.memset(eps_sb[:], EPS)

    # ---- persistent activations ----
    qT_sb = persist.tile([P, S], BF, tag="qT_sb")      # Q^T (+bias), 2 heads
    kT_sb = persist.tile([P, S], BF, tag="kT_sb")      # K^T (+bias)
    v_sb = persist.tile([P, NK, 2 * (HD + 1)], BF, tag="v_sb")  # [V0|1|V1|1]
    attnT_sb = persist.tile([P, S], BF, tag="attnT_sb")

    # ---- projections: Q^T/K^T [hd2, S] = W_shard @ x^T ----
    for w, dst in ((wk_sb, kT_sb), (wq_sb, qT_sb)):
        for j in range(NJ):
            ps = psA.tile([P, JW], F32, tag="mm", name="ps")
            for half in range(JW // 512):
                q0 = j * JW + half * 512
                for c in range(NDM):
                    nc.tensor.matmul(ps[:, half * 512:(half + 1) * 512],
                                     lhsT=w[:, c, :],
                                     rhs=xT_sb[:, c, q0:q0 + 512],
                                     start=(c == 0), stop=(c == NDM - 1))
            nc.vector.tensor_add(dst[:, j * JW:(j + 1) * JW], ps[:],
                                 biasT_sb[:, j * JW:(j + 1) * JW])

    # ---- V in [s, hd] layout: V = x @ Wv_shard^T; ones cols appended ----
    for t in range(NK):
        ts = slice(t * P, (t + 1) * P)
        psv = psA.tile([P, JW], F32, tag="mm", name="psv")
        for c in range(NDM):
            nc.tensor.matmul(psv[:, 0:P], lhsT=xT_sb[:, c, ts],
                             rhs=wv_sb[:, c, :],
                             start=(c == 0), stop=(c == NDM - 1))
        nc.vector.tensor_copy(v_sb[:, t, 0:HD], psv[:, 0:HD])
        nc.vector.tensor_copy(v_sb[:, t, HD + 1:2 * HD + 1], psv[:, HD:2 * HD])
    nc.vector.memset(v_sb[:, :, HD:HD + 1], 1.0)
    nc.vector.memset(v_sb[:, :, 2 * HD + 1:2 * HD + 2], 1.0)

    # AllGather bounce buffers (bf16): in = my heads' attn block,
    # out = [src core, hd-slice, q of block]
    ag_in, ag_out = [], []
    for j in range(NJ):
        ag_in_j = dram.tile([P, JW], BF, tag=f"ag_in_{j}", name=f"ag_in_{j}")
        ag_out_j = dram.tile([NCORES, P, JW], BF, tag=f"ag_out_{j}",
                             name=f"ag_out_{j}", addr_space="Shared")
        ag_in.append(ag_in_j)
        ag_out.append(ag_out_j)

    inv_sqrt_hd = float(1.0 / np.sqrt(HD))
    for j in range(NJ):
        js = slice(j * JW, (j + 1) * JW)
        # ---- attention for this q-superblock, per head ----
        for h in range(HPC):
            hs = slice(h * HD, (h + 1) * HD)
            pv = psPV.tile([HD + 1, JW], F32, tag="pv", name="pv")
            for ki in range(NK):
                ks = slice(ki * P, (ki + 1) * P)
                lg = psA.tile([P, JW], F32, tag="mm", name="lg")
                for half in range(JW // 512):
                    q0 = j * JW + half * 512
                    nc.tensor.matmul(lg[:, half * 512:(half + 1) * 512],
                                     lhsT=kT_sb[hs, ks],
                                     rhs=qT_sb[hs, q0:q0 + 512],
                                     start=True, stop=True)
                pt = ptp.tile([P, JW], BF, tag="pt", name="pt")
                nc.scalar.activation(pt[:], lg[:], AF.Exp, scale=inv_sqrt_hd)
                vcol = slice(h * (HD + 1), (h + 1) * (HD + 1))
                for half in range(JW // 512):
                    nc.tensor.matmul(pv[:, half * 512:(half + 1) * 512],
                                     lhsT=v_sb[:, ki, vcol],
                                     rhs=pt[:, half * 512:(half + 1) * 512],
                                     start=(ki == 0), stop=(ki == NK - 1))
            # drain pv to SBUF immediately (frees the psum bank for the
            # next head), then normalize off the critical path:
            # denom row -> DRAM -> [128,8] spread -> reciprocal -> DRAM ->
            # partition-broadcast -> multiply
            praw = small.tile([HD + 1, JW], F32, tag="praw", name="praw")
            nc.vector.tensor_copy(praw[:], pv[:])
            drec = dram.tile([1, JW], F32, tag="drec", name="drec", bufs=2)
            nc.sync.dma_start(drec[:], praw[HD:HD + 1, :])
            dspread = small.tile([P, JW // P, 1], F32, tag="dspread",
                                 name="dspread")
            nc.sync.dma_start(dspread[:],
                              drec.rearrange("one (u p) -> p u one", p=P))
            rspread = small.tile([P, JW // P, 1], F32, tag="rspread",
                                 name="rspread")
            nc.vector.reciprocal(rspread[:], dspread[:])
            drec2 = dram.tile([1, JW], F32, tag="drec2", name="drec2", bufs=2)
            nc.sync.dma_start(drec2.rearrange("one (u p) -> p u one", p=P),
                              rspread[:])
            rb = small.tile([HD, JW], F32, tag="rb", name="rb")
            nc.sync.dma_start(rb[:], drec2.to_broadcast((HD, JW)))
            if h == 0:
                nc.vector.tensor_tensor(out=attnT_sb[0:HD, js],
                                        in0=praw[0:HD, :], in1=rb[:],
                                        op=ALU.mult)
            else:
                h1t = small.tile([HD, JW], BF, tag="h1t", name="h1t")
                nc.vector.tensor_tensor(out=h1t[:], in0=praw[0:HD, :],
                                        in1=rb[:], op=ALU.mult)
                nc.sync.dma_start(attnT_sb[HD:2 * HD, js], h1t[:])

        # ---- AllGather heads, then pick our token slice dynamically ----
        nc.sync.dma_start(ag_in[j][:], attnT_sb[:, js])
        if FAKE_A2A:
            nc.sync.dma_start(ag_out[j][0, :, :], ag_in[j][:])
        else:
            nc.gpsimd.collective_compute(
                "AllGather", ALU.bypass,
                replica_groups=[list(range(NCORES))],
                ins=[ag_in[j][:].opt()],
                outs=[ag_out[j][:].opt()],
            )
        afull = small.tile([P, NCORES, TOK], BF, tag="afull", name="afull")
        pid = nc.sync.partition_id()
        ag_v = ag_out[j].rearrange("c p (u t) -> p c u t", u=NCORES)
        nc.sync.dma_start(afull[:], ag_v[:, :, bass.ds(pid, 1), :])

        # ---- full output projection for our TOK tokens of block j ----
        po = psA.tile([P, DM], F32, tag="po", name="po", bufs=1)
        for n in range(DM // 512):
            ns = slice(n * 512, (n + 1) * 512)
            for c in range(NDM):
                nc.tensor.matmul(po[:, ns], lhsT=afull[:, c, :],
                                 rhs=woF_sb[:, c, ns],
                                 start=(c == 0), stop=(c == NDM - 1))

        # ---- residual + layernorm ----
        res = small.tile([P, DM], F32, tag="res", name="res")
        nc.vector.tensor_add(res[:], po[:], xres_sb[:, j, :])
        sums = small.tile([P, 1], F32, tag="sums", name="sums")
        nc.vector.reduce_sum(sums[:], res[:], axis=mybir.AxisListType.X)
        neg_mu = small.tile([P, 1], F32, tag="neg_mu", name="neg_mu")
        nc.vector.tensor_scalar_mul(neg_mu[:], sums[:], -1.0 / DM)
        sq = small.tile([P, DM], F32, tag="sq", name="sq")
        sumsq = small.tile([P, 1], F32, tag="sumsq", name="sumsq")
        nc.scalar.activation(sq[:], res[:], AF.Square, accum_out=sumsq[:])
        mu2 = small.tile([P, 1], F32, tag="mu2", name="mu2")
        nc.vector.tensor_tensor(out=mu2[:], in0=neg_mu[:], in1=neg_mu[:],
                                op=ALU.mult)
        var = small.tile([P, 1], F32, tag="var", name="var")
        nc.vector.tensor_scalar_mul(var[:], sumsq[:], 1.0 / DM)
        nc.vector.tensor_tensor(out=var[:], in0=var[:], in1=mu2[:],
                                op=ALU.subtract)
        std = small.tile([P, 1], F32, tag="std", name="std")
        nc.scalar.activation(std[:], var[:], AF.Sqrt, bias=eps_sb[:])
        rstd = small.tile([P, 1], F32, tag="rstd", name="rstd")
        nc.vector.reciprocal(rstd[:], std[:])
        lnb = small.tile([P, 1], F32, tag="lnb", name="lnb")
        nc.vector.tensor_tensor(out=lnb[:], in0=neg_mu[:], in1=rstd[:],
                                op=ALU.mult)
        t1 = small.tile([P, DM], F32, tag="t1", name="t1")
        nc.scalar.activation(t1[:], res[:], AF.Identity, scale=rstd[:],
                             bias=lnb[:])
        t2 = small.tile([P, DM], F32, tag="t2", name="t2")
        nc.vector.tensor_tensor(out=t2[:], in0=t1[:], in1=gammab[:],
                                op=ALU.mult)
        outt = small.tile([P, DM], F32, tag="outt", name="outt")
        nc.vector.tensor_tensor(out=outt[:], in0=t2[:], in1=betab[:],
                                op=ALU.add)
        nc.sync.dma_start(out_d[j * TOK:(j + 1) * TOK, :], outt[:])

    if DEBUG_TAPS:
        taps = {
            "dbg_qT": qT_sb, "dbg_kT": kT_sb,
            "dbg_vaug": v_sb, "dbg_attnT": attnT_sb,
        }
        for nm, t in taps.items():
            d = nc.dram_tensor(nm, list(t.shape), t.dtype,
                               kind="ExternalOutput").ap()
            nc.sync.dma_start(d[:], t[:])
        for j in range(NJ):
            d = nc.dram_tensor(f"dbg_a2a_{j}", list(a2a_out[j].shape), BF,
                               kind="ExternalOutput").ap()
            nc.sync.dma_start(d[:], a2a_out[j][:])

    for pool in (dram, psPV, psA, small, ptp, persist, const):
        pool.release()


_NC_CACHE = None


def _get_program():
    global _NC_CACHE
    if _NC_CACHE is None:
        _NC_CACHE = _build_program()
    return _NC_CACHE


def _token_rows(core):
    """Global token indices owned by `core`, in device output order."""
    rows = []
    for j in range(NJ):
        start = j * JW + core * TOK
        rows.extend(range(start, start + TOK))
    return np.array(rows)


def _prep_inputs(x, static_bias, Wq, Wk, Wv, Wo, ln_gamma, ln_beta):
    bf = ml_dtypes.bfloat16
    x = np.asarray(x, np.float32)
    static_bias = np.asarray(static_bias, np.float32)
    Wq, Wk, Wv, Wo = (np.asarray(w, np.float32) for w in (Wq, Wk, Wv, Wo))
    gamma = np.ascontiguousarray(np.asarray(ln_gamma, np.float32).reshape(1, DM))
    beta = np.ascontiguousarray(np.asarray(ln_beta, np.float32).reshape(1, DM))
    xT = np.ascontiguousarray(x.T).astype(bf)
    woF = np.ascontiguousarray(Wo.T.reshape(NDM, 128, DM)).astype(bf)
    in_maps = []
    for c in range(NCORES):
        hs = slice(c * HD2, (c + 1) * HD2)
        wqT = np.ascontiguousarray(Wq[hs, :].T).astype(bf)
        wkT = np.ascontiguousarray(Wk[hs, :].T).astype(bf)
        wvT = np.ascontiguousarray(Wv[hs, :].T).astype(bf)
        biasT = np.ascontiguousarray(
            static_bias[:, c * HPC:(c + 1) * HPC, :].reshape(S, HD2).T)
        xres = np.ascontiguousarray(x[_token_rows(c), :])
        in_maps.append({
            "xT": xT, "wqT": wqT, "wkT": wkT, "wvT": wvT, "woF": woF,
            "biasT": biasT, "xres": xres, "gamma": gamma, "beta": beta,
        })
    return in_maps


def _assemble(results):
    out = np.empty((S, DM), np.float32)
    for c in range(NCORES):
        out[_token_rows(c), :] = results[c]["out"]
    return out


def kernel(x, static_bias, Wq, Wk, Wv, Wo, ln_gamma, ln_beta, mask=None,
           **_ignored):
    nc = _get_program()
    in_maps = _prep_inputs(x, static_bias, Wq, Wk, Wv, Wo, ln_gamma, ln_beta)
    res = bass_utils.run_bass_kernel_spmd(nc, in_maps,
                                          core_ids=list(range(NCORES)))
    return _assemble(res.results)


if __name__ == "__main__":
    import reference
    inputs = {k: np.asarray(v) for k, v in reference.setup_inputs().items()}
    expected = np.asarray(reference.reference(**inputs))
    actual = kernel(**inputs)
    err = np.abs(actual - expected)
    denom = np.abs(expected).max()
    print("absmax err:", err.max(), "rel:", err.max() / denom)
